# revision 12
# baseline (speedup 1.0000x reference)
"""Trainium2 Bass kernel for nn_CrossAttention (B=4, NQ=NK=1024, D=1024, H=16).

Sharding: 8 cores = 4 batches x 2 head-groups (8 heads each).

v3 design:
  - scores via 64x128 row-tiled matmuls (T0/T8, two heads), [128,1024]
    2-bank PSUM tiles, exp as N=1024 ScalarE ACTIVATEs from PSUM
  - PSUM: 3x[128,1024] score slots (deep ping-pong keeps ScalarE fed)
    + 2x[128,512] chain slots shared by projections / PV / out-proj
  - PV for pair p runs compactly (not exp-gated) during pair p+1's
    score window; V/QK projections of later pairs fill remaining PE slack
  - inputs spread over 5 DMA queues (sync/vector/scalar/tensor HWDGE +
    gpsimd SWDGE), ~93GB/s each; warm-up matmuls on a zero tile keep the
    PE HAM clock-gate at 2.4GHz through the DMA-paced start
  - output fp16, DMAs round-robin all four HWDGE queues; host combines
    the two head-group partials per batch in fp32
"""
import sys

sys.path.insert(0, "/opt/trn_rl_repo")

from contextlib import ExitStack

import numpy as np

import concourse.bass as bass
import concourse.tile as tile
from concourse import bacc, mybir
from concourse.bass_utils import run_bass_kernel_spmd

F32 = mybir.dt.float32
F16 = mybir.dt.float16

B, NQ, NK, D, H, HD = 4, 1024, 1024, 1024, 16, 64
NCORES = 8
HPC = 8          # heads per core
F = HPC * HD     # 512: per-core projection width
KT = D // 128    # 8 k-tiles over D
PAIRS = HPC // 2  # 4 head pairs
TKT = NK // 128  # 8 tiles over key tokens
NCH = NQ // 512  # 2 chunks over query tokens


def _emit(tc):
    nc = tc.nc
    ctx = ExitStack()

    xqT = nc.dram_tensor("xqT", [D, NQ], F16, kind="ExternalInput").ap()
    xkT = nc.dram_tensor("xkT", [D, NK], F16, kind="ExternalInput").ap()
    xvT = nc.dram_tensor("xvT", [D, NK], F16, kind="ExternalInput").ap()
    wq = nc.dram_tensor("wq", [D, F], F16, kind="ExternalInput").ap()
    wk = nc.dram_tensor("wk", [D, F], F16, kind="ExternalInput").ap()
    wv = nc.dram_tensor("wv", [D, F], F16, kind="ExternalInput").ap()
    wo = nc.dram_tensor("wo", [F, D], F16, kind="ExternalInput").ap()
    out = nc.dram_tensor("out", [NQ, D], F16, kind="ExternalOutput").ap()

    wpool = ctx.enter_context(tc.tile_pool(name="wpool", bufs=1))
    qkv = ctx.enter_context(tc.tile_pool(name="qkv", bufs=1))
    xpool = ctx.enter_context(tc.tile_pool(name="xpool", bufs=24))
    expool = ctx.enter_context(tc.tile_pool(name="expool", bufs=22))
    # PSUM: sc 3x[128,1024] (6 banks) + ov 2x[128,512] (2 banks)
    psum = ctx.enter_context(tc.tile_pool(name="psum", bufs=2, space="PSUM"))
    nrm = ctx.enter_context(tc.tile_pool(name="nrm", bufs=1))
    ost = ctx.enter_context(tc.tile_pool(name="ost", bufs=4))

    # ---- persistent weights ----
    wq_sb = wpool.tile([128, KT, F], F16, tag="wq")
    wk_sb = wpool.tile([128, KT, F], F16, tag="wk")
    wv_sb = wpool.tile([128, KT, F], F16, tag="wv")
    wo_sb = wpool.tile([128, PAIRS, D], F16, tag="wo")

    wq_r = wq.rearrange("(k p) f -> p k f", k=KT)
    wk_r = wk.rearrange("(k p) f -> p k f", k=KT)

    # ---- DMA plan: 3 queues (only sync/scalar are HWDGE, gpsimd is SWDGE)
    # sync:   wq-m0, xq k0-7, xv evens, wo p0/p1
    # scalar: wk-m0, xk k0-7, xv odds, wo p2/p3
    # gpsimd: wv, wq/wk-m1, wq/wk-m2/m3
    nc.sync.dma_start(out=wq_sb[:, :, 0:128], in_=wq_r[:, :, 0:128])
    nc.scalar.dma_start(out=wk_sb[:, :, 0:128], in_=wk_r[:, :, 0:128])
    xq_t = [xpool.tile([128, NQ], F16, tag="x", name=f"xq{k}") for k in range(KT)]
    xk_t = [xpool.tile([128, NK], F16, tag="x", name=f"xk{k}") for k in range(KT)]
    xv_t = [xpool.tile([128, NK], F16, tag="x", name=f"xv{k}") for k in range(KT)]
    for k in range(KT):
        nc.sync.dma_start(out=xq_t[k][:], in_=xqT[k * 128:(k + 1) * 128, :])
        nc.scalar.dma_start(out=xk_t[k][:], in_=xkT[k * 128:(k + 1) * 128, :])
    for k in range(KT):
        nc.gpsimd.dma_start(out=wv_sb[:, k, :], in_=wv[k * 128:(k + 1) * 128, :])
    nc.gpsimd.dma_start(out=wq_sb[:, :, 128:256], in_=wq_r[:, :, 128:256])
    nc.gpsimd.dma_start(out=wk_sb[:, :, 128:256], in_=wk_r[:, :, 128:256])
    nc.gpsimd.dma_start(out=wq_sb[:, :, 256:F], in_=wq_r[:, :, 256:F])
    nc.gpsimd.dma_start(out=wk_sb[:, :, 256:F], in_=wk_r[:, :, 256:F])
    for k in range(KT):
        eng = nc.sync if k % 2 == 0 else nc.scalar
        eng.dma_start(out=xv_t[k][:], in_=xvT[k * 128:(k + 1) * 128, :])
    for p in range(PAIRS):
        eng = nc.sync if p < 2 else nc.scalar
        eng.dma_start(out=wo_sb[:, p, :], in_=wo[p * 128:(p + 1) * 128, :])

    # ---- persistent intermediates ----
    qt = [qkv.tile([128, NQ], F16, tag=f"qt{p}", name=f"qt{p}") for p in range(PAIRS)]
    kt = [qkv.tile([128, NK], F16, tag=f"kt{p}", name=f"kt{p}") for p in range(PAIRS)]
    vp_sb = qkv.tile([128, TKT, HPC, HD + 1], F16, tag="vp")  # V + ones col
    att = [qkv.tile([128, NQ], F16, tag=f"att{p}", name=f"att{p}") for p in range(PAIRS)]
    nc.vector.memset(vp_sb[:, :, :, HD:HD + 1], 1.0)

    # warm-up tile: zero fp16, fuels junk matmuls that keep the PE busy
    # (and the HAM clock-gate warm) while input DMAs stream in
    jt = qkv.tile([128, 512], F16, tag="jt")
    nc.vector.memset(jt[:], 0.0)
    jk_ps = psum.tile([128, 512], F32, tag="sc", name="jk", bufs=3)

    def junk(n):
        for _ in range(n):
            nc.tensor.matmul(out=jk_ps[:], lhsT=jt[:, 0:128], rhs=jt[:],
                             start=True, stop=True)

    scale = 1.0 / float(np.sqrt(HD))
    ex = {}

    def emit_qkproj(m, warmup=False):
        """Q^T and K^T projection for head pair m (2 chains per src)."""
        for src, wt, dst, nm in ((xq_t, wq_sb, qt[m], "q"), (xk_t, wk_sb, kt[m], "k")):
            pss = [psum.tile([128, 512], F32, tag="ov",
                             name=f"pj{nm}{m}_{n}") for n in range(NCH)]
            for k in range(KT):
                if warmup:
                    junk(7)
                for n in range(NCH):
                    nc.tensor.matmul(out=pss[n][:],
                                     lhsT=wt[:, k, m * 128:(m + 1) * 128],
                                     rhs=src[k][:, n * 512:(n + 1) * 512],
                                     start=(k == 0), stop=(k == KT - 1))
            for n in range(NCH):
                nc.vector.tensor_copy(out=dst[:, n * 512:(n + 1) * 512],
                                      in_=pss[n][:])

    def emit_scores(p, junk_ov=0):
        """scoresT + exp for pair p: 64x128 row-tiled MMs, N=1024 exps."""
        if junk_ov:
            jk2 = psum.tile([128, 512], F32, tag="ov", name=f"jk2_{p}")
        for t in range(TKT):
            if junk_ov and t < 4:
                for _ in range(junk_ov):
                    nc.tensor.matmul(out=jk2[:], lhsT=jt[:, 0:128], rhs=jt[:],
                                     start=True, stop=True)
            for hh in range(2):
                sc = psum.tile([128, 1024], F32, tag="sc", name=f"sc{p}_{t}_{hh}",
                               bufs=3)
                r0 = hh * 64
                for n in range(NCH):
                    nc.tensor.matmul(
                        out=sc[:, n * 512:(n + 1) * 512],
                        lhsT=kt[p][r0:r0 + 64, t * 128:(t + 1) * 128],
                        rhs=qt[p][r0:r0 + 64, n * 512:(n + 1) * 512],
                        start=True, stop=True)
                e = expool.tile([128, 1024], F16, tag="ex", name=f"ex{p}_{t}_{hh}")
                nc.scalar.activation(out=e[:], in_=sc[:],
                                     func=mybir.ActivationFunctionType.Exp,
                                     scale=scale)
                ex[(p, hh, t)] = e

    def emit_vproj():
        """V projection: 8 kpos-tile chains, 2 at a time."""
        for rnd in range(4):
            psv = [psum.tile([128, 512], F32, tag="ov",
                             name=f"psv{rnd}_{tt}") for tt in range(2)]
            for k in range(KT):
                for tt in range(2):
                    tk = rnd * 2 + tt
                    nc.tensor.matmul(out=psv[tt][:],
                                     lhsT=xv_t[k][:, tk * 128:(tk + 1) * 128],
                                     rhs=wv_sb[:, k, :],
                                     start=(k == 0), stop=(k == KT - 1))
            for tt in range(2):
                tk = rnd * 2 + tt
                nc.vector.tensor_copy(
                    out=vp_sb[:, tk, :, 0:HD],
                    in_=psv[tt][:].rearrange("p (h d) -> p h d", h=HPC))

    def emit_pv(p):
        """PV chains (compact; ex[p] fully available) + normalization."""
        pv_ps = {}
        for hh in range(2):
            h = p * 2 + hh
            pv2 = [psum.tile([65, 512], F32, tag="ov", name=f"pv{p}_{hh}_{n}")
                   for n in range(NCH)]
            for t in range(TKT):
                for n in range(NCH):
                    nc.tensor.matmul(out=pv2[n][:],
                                     lhsT=vp_sb[:, t, h, :],
                                     rhs=ex[(p, hh, t)][:, n * 512:(n + 1) * 512],
                                     start=(t == 0), stop=(t == TKT - 1))
            # normalize this head right away so the ov slots free quickly
            den = nrm.tile([1, NQ], F32, tag="den", name=f"den_{p}_{hh}")
            for n in range(NCH):
                nc.vector.tensor_copy(out=den[0:1, n * 512:(n + 1) * 512],
                                      in_=pv2[n][64:65, :])
            rscr = nrm.tile([1, NQ], F32, tag="rscr", name=f"rs_{p}_{hh}")
            rec = nrm.tile([1, NQ], F32, tag="rec", name=f"rec_{p}_{hh}")
            nc.vector.reciprocal_approx_accurate(out=rec[:], in_=den[:],
                                                 scratch=rscr[:])
            rb = nrm.tile([64, NQ], F32, tag="rb", name=f"rb_{p}_{hh}", bufs=2)
            nc.gpsimd.partition_broadcast(out_ap=rb[:], in_ap=rec[0:1, :],
                                          channels=64)
            if hh == 0:
                for n in range(NCH):
                    nc.vector.tensor_mul(out=att[p][0:64, n * 512:(n + 1) * 512],
                                         in0=pv2[n][0:64, :],
                                         in1=rb[:, n * 512:(n + 1) * 512])
            else:
                tmp = nrm.tile([64, NQ], F16, tag="tmp", name=f"tmp_{p}")
                for n in range(NCH):
                    nc.vector.tensor_mul(out=tmp[:, n * 512:(n + 1) * 512],
                                         in0=pv2[n][0:64, :],
                                         in1=rb[:, n * 512:(n + 1) * 512])
                nc.sync.dma_start(out=att[p][64:128, :], in_=tmp[:])

    # ---- pipelined emission ----
    junk(24)
    emit_qkproj(0, warmup=True)
    emit_scores(0, junk_ov=3)
    emit_qkproj(1)
    emit_scores(1)
    emit_vproj()
    emit_pv(0)
    emit_qkproj(2)
    emit_qkproj(3)
    emit_scores(2)
    emit_pv(1)
    emit_scores(3)
    emit_pv(2)
    emit_pv(3)

    # ---- output projection: blocks of 2 chains, early links hoisted ----
    qn = [(q, n) for q in range(NQ // 128) for n in range(NCH)]
    out_engs = [nc.sync, nc.scalar]
    for blk in range(0, 16, 4):
        group = qn[blk:blk + 4]
        pso = {}
        for gi, (q, n) in enumerate(group):
            pso[(q, n)] = psum.tile([128, 512], F32,
                                    tag=("sc" if gi < 2 else "ov"),
                                    bufs=(3 if gi < 2 else 2),
                                    name=f"pso_{q}_{n}")
            for p4 in range(PAIRS - 1):
                nc.tensor.matmul(out=pso[(q, n)][:],
                                 lhsT=att[p4][:, q * 128:(q + 1) * 128],
                                 rhs=wo_sb[:, p4, n * 512:(n + 1) * 512],
                                 start=(p4 == 0), stop=False)
        for gi, (q, n) in enumerate(group):
            nc.tensor.matmul(out=pso[(q, n)][:],
                             lhsT=att[3][:, q * 128:(q + 1) * 128],
                             rhs=wo_sb[:, 3, n * 512:(n + 1) * 512],
                             start=False, stop=True)
            ot = ost.tile([128, 512], F16, tag="ot", name=f"ot_{q}_{n}")
            nc.vector.tensor_copy(out=ot[:], in_=pso[(q, n)][:])
            out_engs[(blk + gi) % 2].dma_start(
                out=out[q * 128:(q + 1) * 128, n * 512:(n + 1) * 512], in_=ot[:])
    ctx.close()


_NC_CACHE = None


def build():
    global _NC_CACHE
    if _NC_CACHE is None:
        nc = bacc.Bacc("TRN2", target_bir_lowering=False, debug=False,
                       num_devices=NCORES)
        with tile.TileContext(nc) as tc:
            _emit(tc)
        nc.compile()
        _NC_CACHE = nc
    return _NC_CACHE


def make_in_maps(inputs):
    q = np.asarray(inputs["query_tokens"], dtype=np.float32)
    kk = np.asarray(inputs["key_tokens"], dtype=np.float32)
    v = np.asarray(inputs["value_tokens"], dtype=np.float32)
    Wq = np.asarray(inputs["Wq"], dtype=np.float32)
    Wk = np.asarray(inputs["Wk"], dtype=np.float32)
    Wv = np.asarray(inputs["Wv"], dtype=np.float32)
    Wo = np.asarray(inputs["Wo"], dtype=np.float32)

    qT = [np.ascontiguousarray(q[b].T).astype(np.float16) for b in range(B)]
    kT = [np.ascontiguousarray(kk[b].T).astype(np.float16) for b in range(B)]
    vT = [np.ascontiguousarray(v[b].T).astype(np.float16) for b in range(B)]
    wq_g = [np.ascontiguousarray(Wq[:, g * F:(g + 1) * F]).astype(np.float16)
            for g in range(2)]
    wk_g = [np.ascontiguousarray(Wk[:, g * F:(g + 1) * F]).astype(np.float16)
            for g in range(2)]
    wv_g = [np.ascontiguousarray(Wv[:, g * F:(g + 1) * F]).astype(np.float16)
            for g in range(2)]
    wo_g = [np.ascontiguousarray(Wo[g * F:(g + 1) * F, :]).astype(np.float16)
            for g in range(2)]

    in_maps = []
    for c in range(NCORES):
        b, g = c // 2, c % 2
        in_maps.append({
            "xqT": qT[b], "xkT": kT[b], "xvT": vT[b],
            "wq": wq_g[g], "wk": wk_g[g], "wv": wv_g[g], "wo": wo_g[g],
        })
    return in_maps


def combine(results, bo):
    out = np.zeros((B, NQ, D), dtype=np.float32)
    for c in range(NCORES):
        out[c // 2] += results[c]["out"].astype(np.float32)
    out += np.asarray(bo, dtype=np.float32)[None, None, :]
    return out


def kernel(**inputs):
    nc = build()
    in_maps = make_in_maps(inputs)
    res = run_bass_kernel_spmd(nc, in_maps, list(range(NCORES)))
    return combine(res.results, inputs["bo"])


# revision 14
# speedup vs baseline: 1.0423x; 1.0423x over previous
"""Trainium2 Bass kernel for nn_CrossAttention (B=4, NQ=NK=1024, D=1024, H=16).

Sharding: 8 cores = 4 batches x 2 head-groups (8 heads each).

v3 design:
  - scores via 64x128 row-tiled matmuls (T0/T8, two heads), [128,1024]
    2-bank PSUM tiles, exp as N=1024 ScalarE ACTIVATEs from PSUM
  - PSUM: 3x[128,1024] score slots (deep ping-pong keeps ScalarE fed)
    + 2x[128,512] chain slots shared by projections / PV / out-proj
  - PV for pair p runs compactly (not exp-gated) during pair p+1's
    score window; V/QK projections of later pairs fill remaining PE slack
  - inputs spread over 5 DMA queues (sync/vector/scalar/tensor HWDGE +
    gpsimd SWDGE), ~93GB/s each; warm-up matmuls on a zero tile keep the
    PE HAM clock-gate at 2.4GHz through the DMA-paced start
  - output fp16, DMAs round-robin all four HWDGE queues; host combines
    the two head-group partials per batch in fp32
"""
import sys

sys.path.insert(0, "/opt/trn_rl_repo")

from contextlib import ExitStack

import numpy as np

import concourse.bass as bass
import concourse.tile as tile
from concourse import bacc, mybir
from concourse.bass_utils import run_bass_kernel_spmd

F32 = mybir.dt.float32
F16 = mybir.dt.float16

B, NQ, NK, D, H, HD = 4, 1024, 1024, 1024, 16, 64
NCORES = 8
HPC = 8          # heads per core
F = HPC * HD     # 512: per-core projection width
KT = D // 128    # 8 k-tiles over D
PAIRS = HPC // 2  # 4 head pairs
TKT = NK // 128  # 8 tiles over key tokens
NCH = NQ // 512  # 2 chunks over query tokens


def _emit(tc):
    nc = tc.nc
    ctx = ExitStack()

    xqT = nc.dram_tensor("xqT", [D, NQ], F16, kind="ExternalInput").ap()
    xkT = nc.dram_tensor("xkT", [D, NK], F16, kind="ExternalInput").ap()
    xvT = nc.dram_tensor("xvT", [D, NK], F16, kind="ExternalInput").ap()
    wq = nc.dram_tensor("wq", [D, F], F16, kind="ExternalInput").ap()
    wk = nc.dram_tensor("wk", [D, F], F16, kind="ExternalInput").ap()
    wv = nc.dram_tensor("wv", [D, F], F16, kind="ExternalInput").ap()
    wo = nc.dram_tensor("wo", [F, D], F16, kind="ExternalInput").ap()
    out = nc.dram_tensor("out", [NQ, D], F16, kind="ExternalOutput").ap()

    wpool = ctx.enter_context(tc.tile_pool(name="wpool", bufs=1))
    qkv = ctx.enter_context(tc.tile_pool(name="qkv", bufs=1))
    xpool = ctx.enter_context(tc.tile_pool(name="xpool", bufs=24))
    expool = ctx.enter_context(tc.tile_pool(name="expool", bufs=22))
    # PSUM: sc 3x[128,1024] (6 banks) + ov 2x[128,512] (2 banks)
    psum = ctx.enter_context(tc.tile_pool(name="psum", bufs=2, space="PSUM"))
    nrm = ctx.enter_context(tc.tile_pool(name="nrm", bufs=1))
    ost = ctx.enter_context(tc.tile_pool(name="ost", bufs=4))

    # ---- persistent weights ----
    wq_sb = wpool.tile([128, KT, F], F16, tag="wq")
    wk_sb = wpool.tile([128, KT, F], F16, tag="wk")
    wv_sb = wpool.tile([128, KT, F], F16, tag="wv")
    wo_sb = wpool.tile([128, PAIRS, D], F16, tag="wo")

    wq_r = wq.rearrange("(k p) f -> p k f", k=KT)
    wk_r = wk.rearrange("(k p) f -> p k f", k=KT)

    # ---- DMA plan: 3 queues (only sync/scalar are HWDGE, gpsimd is SWDGE)
    # sync:   xq k0-7, wq full k0-7, xv k0-7
    # scalar: xk k0-7, wk full k0-7, wo
    # gpsimd: wq-m0 slices, wk-m0 slices, wv
    xq_t = [xpool.tile([128, NQ], F16, tag="x", name=f"xq{k}") for k in range(KT)]
    xk_t = [xpool.tile([128, NK], F16, tag="x", name=f"xk{k}") for k in range(KT)]
    xv_t = [xpool.tile([128, NK], F16, tag="x", name=f"xv{k}") for k in range(KT)]
    for k in range(KT):
        nc.sync.dma_start(out=xq_t[k][:], in_=xqT[k * 128:(k + 1) * 128, :])
        nc.scalar.dma_start(out=xk_t[k][:], in_=xkT[k * 128:(k + 1) * 128, :])
        nc.gpsimd.dma_start(out=wq_sb[:, k, 0:128],
                            in_=wq[k * 128:(k + 1) * 128, 0:128])
    for k in range(KT):
        nc.gpsimd.dma_start(out=wk_sb[:, k, 0:128],
                            in_=wk[k * 128:(k + 1) * 128, 0:128])
    for k in range(KT):
        nc.sync.dma_start(out=wq_sb[:, k, 128:F],
                          in_=wq[k * 128:(k + 1) * 128, 128:F])
        nc.scalar.dma_start(out=wk_sb[:, k, 128:F],
                            in_=wk[k * 128:(k + 1) * 128, 128:F])
        nc.gpsimd.dma_start(out=wv_sb[:, k, :], in_=wv[k * 128:(k + 1) * 128, :])
    for p in range(PAIRS):
        nc.scalar.dma_start(out=wo_sb[:, p, :], in_=wo[p * 128:(p + 1) * 128, :])
    for k in range(KT):
        nc.sync.dma_start(out=xv_t[k][:], in_=xvT[k * 128:(k + 1) * 128, :])

    # ---- persistent intermediates ----
    qt = [qkv.tile([128, NQ], F16, tag=f"qt{p}", name=f"qt{p}") for p in range(PAIRS)]
    kt = [qkv.tile([128, NK], F16, tag=f"kt{p}", name=f"kt{p}") for p in range(PAIRS)]
    vp_sb = qkv.tile([128, TKT, HPC, HD + 1], F16, tag="vp")  # V + ones col
    att = [qkv.tile([128, NQ], F16, tag=f"att{p}", name=f"att{p}") for p in range(PAIRS)]
    nc.vector.memset(vp_sb[:, :, :, HD:HD + 1], 1.0)

    # warm-up tile: zero fp16, fuels junk matmuls that keep the PE busy
    # (and the HAM clock-gate warm) while input DMAs stream in
    jt = qkv.tile([128, 512], F16, tag="jt")
    nc.vector.memset(jt[:], 0.0)
    jk_ps = psum.tile([128, 512], F32, tag="sc", name="jk", bufs=3)

    def junk(n):
        for _ in range(n):
            nc.tensor.matmul(out=jk_ps[:], lhsT=jt[:, 0:128], rhs=jt[:],
                             start=True, stop=True)

    scale = 1.0 / float(np.sqrt(HD))
    ex = {}

    def emit_qkproj(m, warmup=False):
        """Q^T and K^T projection for head pair m (2 chains per src)."""
        for src, wt, dst, nm in ((xq_t, wq_sb, qt[m], "q"), (xk_t, wk_sb, kt[m], "k")):
            pss = [psum.tile([128, 512], F32, tag="ov",
                             name=f"pj{nm}{m}_{n}") for n in range(NCH)]
            for k in range(KT):
                if warmup:
                    junk(7)
                for n in range(NCH):
                    nc.tensor.matmul(out=pss[n][:],
                                     lhsT=wt[:, k, m * 128:(m + 1) * 128],
                                     rhs=src[k][:, n * 512:(n + 1) * 512],
                                     start=(k == 0), stop=(k == KT - 1))
            for n in range(NCH):
                nc.vector.tensor_copy(out=dst[:, n * 512:(n + 1) * 512],
                                      in_=pss[n][:])

    def emit_scores(p, junk_ov=0):
        """scoresT + exp for pair p: 64x128 row-tiled MMs, N=1024 exps."""
        if junk_ov:
            jk2 = psum.tile([128, 512], F32, tag="ov", name=f"jk2_{p}")
        for t in range(TKT):
            if junk_ov and t < 4:
                for _ in range(junk_ov):
                    nc.tensor.matmul(out=jk2[:], lhsT=jt[:, 0:128], rhs=jt[:],
                                     start=True, stop=True)
            for hh in range(2):
                sc = psum.tile([128, 1024], F32, tag="sc", name=f"sc{p}_{t}_{hh}",
                               bufs=3)
                r0 = hh * 64
                for n in range(NCH):
                    nc.tensor.matmul(
                        out=sc[:, n * 512:(n + 1) * 512],
                        lhsT=kt[p][r0:r0 + 64, t * 128:(t + 1) * 128],
                        rhs=qt[p][r0:r0 + 64, n * 512:(n + 1) * 512],
                        start=True, stop=True)
                e = expool.tile([128, 1024], F16, tag="ex", name=f"ex{p}_{t}_{hh}")
                nc.scalar.activation(out=e[:], in_=sc[:],
                                     func=mybir.ActivationFunctionType.Exp,
                                     scale=scale)
                ex[(p, hh, t)] = e

    def emit_vproj():
        """V projection: 8 kpos-tile chains, 2 at a time."""
        for rnd in range(4):
            psv = [psum.tile([128, 512], F32, tag="ov",
                             name=f"psv{rnd}_{tt}") for tt in range(2)]
            for k in range(KT):
                for tt in range(2):
                    tk = rnd * 2 + tt
                    nc.tensor.matmul(out=psv[tt][:],
                                     lhsT=xv_t[k][:, tk * 128:(tk + 1) * 128],
                                     rhs=wv_sb[:, k, :],
                                     start=(k == 0), stop=(k == KT - 1))
            for tt in range(2):
                tk = rnd * 2 + tt
                nc.vector.tensor_copy(
                    out=vp_sb[:, tk, :, 0:HD],
                    in_=psv[tt][:].rearrange("p (h d) -> p h d", h=HPC))

    def emit_pv(p):
        """PV chains (compact; ex[p] fully available) + normalization."""
        pv_ps = {}
        for hh in range(2):
            h = p * 2 + hh
            pv2 = [psum.tile([65, 512], F32, tag="ov", name=f"pv{p}_{hh}_{n}")
                   for n in range(NCH)]
            for t in range(TKT):
                for n in range(NCH):
                    nc.tensor.matmul(out=pv2[n][:],
                                     lhsT=vp_sb[:, t, h, :],
                                     rhs=ex[(p, hh, t)][:, n * 512:(n + 1) * 512],
                                     start=(t == 0), stop=(t == TKT - 1))
            # normalize this head right away so the ov slots free quickly
            den = nrm.tile([1, NQ], F32, tag="den", name=f"den_{p}_{hh}")
            for n in range(NCH):
                nc.vector.tensor_copy(out=den[0:1, n * 512:(n + 1) * 512],
                                      in_=pv2[n][64:65, :])
            rscr = nrm.tile([1, NQ], F32, tag="rscr", name=f"rs_{p}_{hh}")
            rec = nrm.tile([1, NQ], F32, tag="rec", name=f"rec_{p}_{hh}")
            nc.vector.reciprocal_approx_accurate(out=rec[:], in_=den[:],
                                                 scratch=rscr[:])
            rb = nrm.tile([64, NQ], F32, tag="rb", name=f"rb_{p}_{hh}", bufs=2)
            nc.gpsimd.partition_broadcast(out_ap=rb[:], in_ap=rec[0:1, :],
                                          channels=64)
            if hh == 0:
                for n in range(NCH):
                    nc.vector.tensor_mul(out=att[p][0:64, n * 512:(n + 1) * 512],
                                         in0=pv2[n][0:64, :],
                                         in1=rb[:, n * 512:(n + 1) * 512])
            else:
                tmp = nrm.tile([64, NQ], F16, tag="tmp", name=f"tmp_{p}")
                for n in range(NCH):
                    nc.vector.tensor_mul(out=tmp[:, n * 512:(n + 1) * 512],
                                         in0=pv2[n][0:64, :],
                                         in1=rb[:, n * 512:(n + 1) * 512])
                nc.sync.dma_start(out=att[p][64:128, :], in_=tmp[:])

    # ---- pipelined emission ----
    junk(24)
    emit_qkproj(0, warmup=True)
    emit_scores(0, junk_ov=3)
    emit_qkproj(1)
    emit_qkproj(2)
    emit_qkproj(3)
    emit_scores(1)
    emit_vproj()
    emit_scores(2)
    emit_pv(0)
    emit_pv(1)
    emit_scores(3)
    emit_pv(2)
    emit_pv(3)

    # ---- output projection: blocks of 2 chains, early links hoisted ----
    qn = [(q, n) for q in range(NQ // 128) for n in range(NCH)]
    out_engs = [nc.sync, nc.scalar]
    for blk in range(0, 16, 4):
        group = qn[blk:blk + 4]
        pso = {}
        for gi, (q, n) in enumerate(group):
            pso[(q, n)] = psum.tile([128, 512], F32,
                                    tag=("sc" if gi < 2 else "ov"),
                                    bufs=(3 if gi < 2 else 2),
                                    name=f"pso_{q}_{n}")
            for p4 in range(PAIRS - 1):
                nc.tensor.matmul(out=pso[(q, n)][:],
                                 lhsT=att[p4][:, q * 128:(q + 1) * 128],
                                 rhs=wo_sb[:, p4, n * 512:(n + 1) * 512],
                                 start=(p4 == 0), stop=False)
        for gi, (q, n) in enumerate(group):
            nc.tensor.matmul(out=pso[(q, n)][:],
                             lhsT=att[3][:, q * 128:(q + 1) * 128],
                             rhs=wo_sb[:, 3, n * 512:(n + 1) * 512],
                             start=False, stop=True)
            ot = ost.tile([128, 512], F16, tag="ot", name=f"ot_{q}_{n}")
            nc.vector.tensor_copy(out=ot[:], in_=pso[(q, n)][:])
            out_engs[(blk + gi) % 2].dma_start(
                out=out[q * 128:(q + 1) * 128, n * 512:(n + 1) * 512], in_=ot[:])
    ctx.close()


_NC_CACHE = None


def build():
    global _NC_CACHE
    if _NC_CACHE is None:
        nc = bacc.Bacc("TRN2", target_bir_lowering=False, debug=False,
                       num_devices=NCORES)
        with tile.TileContext(nc) as tc:
            _emit(tc)
        nc.compile()
        _NC_CACHE = nc
    return _NC_CACHE


def make_in_maps(inputs):
    q = np.asarray(inputs["query_tokens"], dtype=np.float32)
    kk = np.asarray(inputs["key_tokens"], dtype=np.float32)
    v = np.asarray(inputs["value_tokens"], dtype=np.float32)
    Wq = np.asarray(inputs["Wq"], dtype=np.float32)
    Wk = np.asarray(inputs["Wk"], dtype=np.float32)
    Wv = np.asarray(inputs["Wv"], dtype=np.float32)
    Wo = np.asarray(inputs["Wo"], dtype=np.float32)

    qT = [np.ascontiguousarray(q[b].T).astype(np.float16) for b in range(B)]
    kT = [np.ascontiguousarray(kk[b].T).astype(np.float16) for b in range(B)]
    vT = [np.ascontiguousarray(v[b].T).astype(np.float16) for b in range(B)]
    wq_g = [np.ascontiguousarray(Wq[:, g * F:(g + 1) * F]).astype(np.float16)
            for g in range(2)]
    wk_g = [np.ascontiguousarray(Wk[:, g * F:(g + 1) * F]).astype(np.float16)
            for g in range(2)]
    wv_g = [np.ascontiguousarray(Wv[:, g * F:(g + 1) * F]).astype(np.float16)
            for g in range(2)]
    wo_g = [np.ascontiguousarray(Wo[g * F:(g + 1) * F, :]).astype(np.float16)
            for g in range(2)]

    in_maps = []
    for c in range(NCORES):
        b, g = c // 2, c % 2
        in_maps.append({
            "xqT": qT[b], "xkT": kT[b], "xvT": vT[b],
            "wq": wq_g[g], "wk": wk_g[g], "wv": wv_g[g], "wo": wo_g[g],
        })
    return in_maps


def combine(results, bo):
    out = np.zeros((B, NQ, D), dtype=np.float32)
    for c in range(NCORES):
        out[c // 2] += results[c]["out"].astype(np.float32)
    out += np.asarray(bo, dtype=np.float32)[None, None, :]
    return out


def kernel(**inputs):
    nc = build()
    in_maps = make_in_maps(inputs)
    res = run_bass_kernel_spmd(nc, in_maps, list(range(NCORES)))
    return combine(res.results, inputs["bo"])


# revision 19
# speedup vs baseline: 1.0458x; 1.0033x over previous
"""Trainium2 Bass kernel for nn_CrossAttention (B=4, NQ=NK=1024, D=1024, H=16).

Sharding: 8 cores = 4 batches x 2 head-groups (8 heads each).

v3 design:
  - scores via 64x128 row-tiled matmuls (T0/T8, two heads), [128,1024]
    2-bank PSUM tiles, exp as N=1024 ScalarE ACTIVATEs from PSUM
  - PSUM: 3x[128,1024] score slots (deep ping-pong keeps ScalarE fed)
    + 2x[128,512] chain slots shared by projections / PV / out-proj
  - PV for pair p runs compactly (not exp-gated) during pair p+1's
    score window; V/QK projections of later pairs fill remaining PE slack
  - inputs spread over 5 DMA queues (sync/vector/scalar/tensor HWDGE +
    gpsimd SWDGE), ~93GB/s each; warm-up matmuls on a zero tile keep the
    PE HAM clock-gate at 2.4GHz through the DMA-paced start
  - output fp16, DMAs round-robin all four HWDGE queues; host combines
    the two head-group partials per batch in fp32
"""
import sys

sys.path.insert(0, "/opt/trn_rl_repo")

from contextlib import ExitStack

import numpy as np

import concourse.bass as bass
import concourse.tile as tile
from concourse import bacc, mybir
from concourse.bass_utils import run_bass_kernel_spmd

F32 = mybir.dt.float32
F16 = mybir.dt.float16

B, NQ, NK, D, H, HD = 4, 1024, 1024, 1024, 16, 64
NCORES = 8
HPC = 8          # heads per core
F = HPC * HD     # 512: per-core projection width
KT = D // 128    # 8 k-tiles over D
PAIRS = HPC // 2  # 4 head pairs
TKT = NK // 128  # 8 tiles over key tokens
NCH = NQ // 512  # 2 chunks over query tokens


def _emit(tc):
    nc = tc.nc
    ctx = ExitStack()

    xqT = nc.dram_tensor("xqT", [D, NQ], F16, kind="ExternalInput").ap()
    xkT = nc.dram_tensor("xkT", [D, NK], F16, kind="ExternalInput").ap()
    xvT = nc.dram_tensor("xvT", [D, NK], F16, kind="ExternalInput").ap()
    wq = nc.dram_tensor("wq", [D, F], F16, kind="ExternalInput").ap()
    wk = nc.dram_tensor("wk", [D, F], F16, kind="ExternalInput").ap()
    wv = nc.dram_tensor("wv", [D, F], F16, kind="ExternalInput").ap()
    wo = nc.dram_tensor("wo", [F, D], F16, kind="ExternalInput").ap()
    out = nc.dram_tensor("out", [NQ, D], F16, kind="ExternalOutput").ap()

    wpool = ctx.enter_context(tc.tile_pool(name="wpool", bufs=1))
    qkv = ctx.enter_context(tc.tile_pool(name="qkv", bufs=1))
    xpool = ctx.enter_context(tc.tile_pool(name="xpool", bufs=24))
    expool = ctx.enter_context(tc.tile_pool(name="expool", bufs=22))
    # PSUM: sc 3x[128,1024] (6 banks) + ov 2x[128,512] (2 banks)
    psum = ctx.enter_context(tc.tile_pool(name="psum", bufs=2, space="PSUM"))
    nrm = ctx.enter_context(tc.tile_pool(name="nrm", bufs=1))
    ost = ctx.enter_context(tc.tile_pool(name="ost", bufs=4))

    # ---- persistent weights ----
    wq_sb = wpool.tile([128, KT, F], F16, tag="wq")
    wk_sb = wpool.tile([128, KT, F], F16, tag="wk")
    wv_sb = wpool.tile([128, KT, F], F16, tag="wv")
    wo_sb = wpool.tile([128, PAIRS, D], F16, tag="wo")

    wq_r = wq.rearrange("(k p) f -> p k f", k=KT)
    wk_r = wk.rearrange("(k p) f -> p k f", k=KT)

    # ---- DMA plan: 3 queues (only sync/scalar are HWDGE, gpsimd is SWDGE)
    # sync:   xq k0-7, wq full k0-7, xv k0-7
    # scalar: xk k0-7, wk full k0-7, wo
    # gpsimd: wq-m0 slices, wk-m0 slices, wv
    xq_t = [xpool.tile([128, NQ], F16, tag="x", name=f"xq{k}") for k in range(KT)]
    xk_t = [xpool.tile([128, NK], F16, tag="x", name=f"xk{k}") for k in range(KT)]
    xv_t = [xpool.tile([128, NK], F16, tag="x", name=f"xv{k}") for k in range(KT)]
    for k in range(KT):
        nc.sync.dma_start(out=xq_t[k][:], in_=xqT[k * 128:(k + 1) * 128, :])
        nc.scalar.dma_start(out=xk_t[k][:], in_=xkT[k * 128:(k + 1) * 128, :])
        nc.gpsimd.dma_start(out=wq_sb[:, k, 0:128],
                            in_=wq[k * 128:(k + 1) * 128, 0:128])
    for k in range(KT):
        nc.gpsimd.dma_start(out=wk_sb[:, k, 0:128],
                            in_=wk[k * 128:(k + 1) * 128, 0:128])
    for k in range(KT):
        nc.sync.dma_start(out=wq_sb[:, k, 128:F],
                          in_=wq[k * 128:(k + 1) * 128, 128:F])
        nc.scalar.dma_start(out=wk_sb[:, k, 128:F],
                            in_=wk[k * 128:(k + 1) * 128, 128:F])
        nc.gpsimd.dma_start(out=wv_sb[:, k, :], in_=wv[k * 128:(k + 1) * 128, :])
    for p in range(PAIRS):
        nc.scalar.dma_start(out=wo_sb[:, p, :], in_=wo[p * 128:(p + 1) * 128, :])
    for k in range(KT):
        nc.sync.dma_start(out=xv_t[k][:], in_=xvT[k * 128:(k + 1) * 128, :])

    # ---- persistent intermediates ----
    qt = [qkv.tile([128, NQ], F16, tag=f"qt{p}", name=f"qt{p}") for p in range(PAIRS)]
    kt = [qkv.tile([128, NK], F16, tag=f"kt{p}", name=f"kt{p}") for p in range(PAIRS)]
    vp_sb = qkv.tile([128, TKT, HPC, HD + 1], F16, tag="vp")  # V + ones col
    att = [qkv.tile([128, NQ], F16, tag=f"att{p}", name=f"att{p}") for p in range(PAIRS)]
    nc.vector.memset(vp_sb[:, :, :, HD:HD + 1], 1.0)

    # warm-up tile: zero fp16, fuels junk matmuls that keep the PE busy
    # (and the HAM clock-gate warm) while input DMAs stream in
    jt = qkv.tile([128, 512], F16, tag="jt")
    nc.vector.memset(jt[:], 0.0)
    jk_ps = psum.tile([128, 512], F32, tag="sc", name="jk", bufs=3)

    def junk(n):
        for _ in range(n):
            nc.tensor.matmul(out=jk_ps[:], lhsT=jt[:, 0:128], rhs=jt[:],
                             start=True, stop=True)

    def junk_batch(n):
        # reuses the same jk_ps (tag sc): costs one score slot until the
        # last junk matmul retires, then all three slots serve the exps
        for _ in range(n):
            nc.tensor.matmul(out=jk_ps[:], lhsT=jt[:, 0:128], rhs=jt[:],
                             start=True, stop=True)

    scale = 1.0 / float(np.sqrt(HD))
    ex = {}

    def emit_qkproj(m, warmup=False):
        """Q^T and K^T projection for head pair m (2 chains per src)."""
        for src, wt, dst, nm in ((xq_t, wq_sb, qt[m], "q"), (xk_t, wk_sb, kt[m], "k")):
            pss = [psum.tile([128, 512], F32, tag="ov",
                             name=f"pj{nm}{m}_{n}") for n in range(NCH)]
            for k in range(KT):
                if warmup:
                    junk(3)
                for n in range(NCH):
                    nc.tensor.matmul(out=pss[n][:],
                                     lhsT=wt[:, k, m * 128:(m + 1) * 128],
                                     rhs=src[k][:, n * 512:(n + 1) * 512],
                                     start=(k == 0), stop=(k == KT - 1))
            for n in range(NCH):
                nc.vector.tensor_copy(out=dst[:, n * 512:(n + 1) * 512],
                                      in_=pss[n][:])

    def emit_scores(p, junk_sc=0):
        """scoresT + exp for pair p: 64x128 row-tiled MMs, N=1024 exps."""
        for t in range(TKT):
            if junk_sc:
                junk_batch(junk_sc)
            for hh in range(2):
                sc = psum.tile([128, 1024], F32, tag="sc", name=f"sc{p}_{t}_{hh}",
                               bufs=3)
                r0 = hh * 64
                for n in range(NCH):
                    nc.tensor.matmul(
                        out=sc[:, n * 512:(n + 1) * 512],
                        lhsT=kt[p][r0:r0 + 64, t * 128:(t + 1) * 128],
                        rhs=qt[p][r0:r0 + 64, n * 512:(n + 1) * 512],
                        start=True, stop=True)
                e = expool.tile([128, 1024], F16, tag="ex", name=f"ex{p}_{t}_{hh}")
                nc.scalar.activation(out=e[:], in_=sc[:],
                                     func=mybir.ActivationFunctionType.Exp,
                                     scale=scale)
                ex[(p, hh, t)] = e

    def emit_vproj():
        """V projection: 8 kpos-tile chains, 2 at a time."""
        for rnd in range(4):
            psv = [psum.tile([128, 512], F32, tag="ov",
                             name=f"psv{rnd}_{tt}") for tt in range(2)]
            for k in range(KT):
                for tt in range(2):
                    tk = rnd * 2 + tt
                    nc.tensor.matmul(out=psv[tt][:],
                                     lhsT=xv_t[k][:, tk * 128:(tk + 1) * 128],
                                     rhs=wv_sb[:, k, :],
                                     start=(k == 0), stop=(k == KT - 1))
            for tt in range(2):
                tk = rnd * 2 + tt
                nc.vector.tensor_copy(
                    out=vp_sb[:, tk, :, 0:HD],
                    in_=psv[tt][:].rearrange("p (h d) -> p h d", h=HPC))

    def emit_pv(p):
        """PV chains (compact; ex[p] fully available) + normalization."""
        pv_ps = {}
        for hh in range(2):
            h = p * 2 + hh
            pv2 = [psum.tile([65, 512], F32, tag="ov", name=f"pv{p}_{hh}_{n}")
                   for n in range(NCH)]
            for t in range(TKT):
                for n in range(NCH):
                    nc.tensor.matmul(out=pv2[n][:],
                                     lhsT=vp_sb[:, t, h, :],
                                     rhs=ex[(p, hh, t)][:, n * 512:(n + 1) * 512],
                                     start=(t == 0), stop=(t == TKT - 1))
            # normalize this head right away so the ov slots free quickly
            den = nrm.tile([1, NQ], F32, tag="den", name=f"den_{p}_{hh}")
            for n in range(NCH):
                nc.vector.tensor_copy(out=den[0:1, n * 512:(n + 1) * 512],
                                      in_=pv2[n][64:65, :])
            rscr = nrm.tile([1, NQ], F32, tag="rscr", name=f"rs_{p}_{hh}")
            rec = nrm.tile([1, NQ], F32, tag="rec", name=f"rec_{p}_{hh}")
            nc.vector.reciprocal_approx_accurate(out=rec[:], in_=den[:],
                                                 scratch=rscr[:])
            rb = nrm.tile([64, NQ], F32, tag="rb", name=f"rb_{p}_{hh}", bufs=2)
            nc.gpsimd.partition_broadcast(out_ap=rb[:], in_ap=rec[0:1, :],
                                          channels=64)
            if hh == 0:
                for n in range(NCH):
                    nc.vector.tensor_mul(out=att[p][0:64, n * 512:(n + 1) * 512],
                                         in0=pv2[n][0:64, :],
                                         in1=rb[:, n * 512:(n + 1) * 512])
            else:
                tmp = nrm.tile([64, NQ], F16, tag="tmp", name=f"tmp_{p}")
                for n in range(NCH):
                    nc.vector.tensor_mul(out=tmp[:, n * 512:(n + 1) * 512],
                                         in0=pv2[n][0:64, :],
                                         in1=rb[:, n * 512:(n + 1) * 512])
                nc.sync.dma_start(out=att[p][64:128, :], in_=tmp[:])

    # ---- pipelined emission ----
    junk(12)
    emit_qkproj(0, warmup=True)
    emit_scores(0, junk_sc=6)
    emit_qkproj(1)
    emit_scores(1)
    emit_qkproj(2)
    emit_vproj()
    emit_scores(2)
    emit_qkproj(3)
    emit_pv(0)
    emit_pv(1)
    emit_scores(3)
    emit_pv(2)
    emit_pv(3)

    # ---- output projection: 2 q-rows (4 chains) per block, early links
    # hoisted ahead of the att[3]-gated finals; [128,1024] out DMAs ----
    out_engs = [nc.sync, nc.scalar]
    for blk in range(4):
        qs = (2 * blk, 2 * blk + 1)
        pso = {}
        for gi, q in enumerate(qs):
            for n in range(NCH):
                pso[(q, n)] = psum.tile([128, 512], F32,
                                        tag=("sc" if gi == 0 else "ov"),
                                        bufs=(3 if gi == 0 else 2),
                                        name=f"pso_{q}_{n}")
                for p4 in range(PAIRS - 1):
                    nc.tensor.matmul(out=pso[(q, n)][:],
                                     lhsT=att[p4][:, q * 128:(q + 1) * 128],
                                     rhs=wo_sb[:, p4, n * 512:(n + 1) * 512],
                                     start=(p4 == 0), stop=False)
        for gi, q in enumerate(qs):
            ot = ost.tile([128, 1024], F16, tag="ot", name=f"ot_{q}")
            for n in range(NCH):
                nc.tensor.matmul(out=pso[(q, n)][:],
                                 lhsT=att[3][:, q * 128:(q + 1) * 128],
                                 rhs=wo_sb[:, 3, n * 512:(n + 1) * 512],
                                 start=False, stop=True)
                nc.vector.tensor_copy(out=ot[:, n * 512:(n + 1) * 512],
                                      in_=pso[(q, n)][:])
            out_engs[(2 * blk + gi) % 2].dma_start(
                out=out[q * 128:(q + 1) * 128, :], in_=ot[:])
    ctx.close()


_NC_CACHE = None


def build():
    global _NC_CACHE
    if _NC_CACHE is None:
        nc = bacc.Bacc("TRN2", target_bir_lowering=False, debug=False,
                       num_devices=NCORES)
        with tile.TileContext(nc) as tc:
            _emit(tc)
        nc.compile()
        _NC_CACHE = nc
    return _NC_CACHE


def make_in_maps(inputs):
    q = np.asarray(inputs["query_tokens"], dtype=np.float32)
    kk = np.asarray(inputs["key_tokens"], dtype=np.float32)
    v = np.asarray(inputs["value_tokens"], dtype=np.float32)
    Wq = np.asarray(inputs["Wq"], dtype=np.float32)
    Wk = np.asarray(inputs["Wk"], dtype=np.float32)
    Wv = np.asarray(inputs["Wv"], dtype=np.float32)
    Wo = np.asarray(inputs["Wo"], dtype=np.float32)

    qT = [np.ascontiguousarray(q[b].T).astype(np.float16) for b in range(B)]
    kT = [np.ascontiguousarray(kk[b].T).astype(np.float16) for b in range(B)]
    vT = [np.ascontiguousarray(v[b].T).astype(np.float16) for b in range(B)]
    wq_g = [np.ascontiguousarray(Wq[:, g * F:(g + 1) * F]).astype(np.float16)
            for g in range(2)]
    wk_g = [np.ascontiguousarray(Wk[:, g * F:(g + 1) * F]).astype(np.float16)
            for g in range(2)]
    wv_g = [np.ascontiguousarray(Wv[:, g * F:(g + 1) * F]).astype(np.float16)
            for g in range(2)]
    wo_g = [np.ascontiguousarray(Wo[g * F:(g + 1) * F, :]).astype(np.float16)
            for g in range(2)]

    in_maps = []
    for c in range(NCORES):
        b, g = c // 2, c % 2
        in_maps.append({
            "xqT": qT[b], "xkT": kT[b], "xvT": vT[b],
            "wq": wq_g[g], "wk": wk_g[g], "wv": wv_g[g], "wo": wo_g[g],
        })
    return in_maps


def combine(results, bo):
    out = np.zeros((B, NQ, D), dtype=np.float32)
    for c in range(NCORES):
        out[c // 2] += results[c]["out"].astype(np.float32)
    out += np.asarray(bo, dtype=np.float32)[None, None, :]
    return out


def kernel(**inputs):
    nc = build()
    in_maps = make_in_maps(inputs)
    res = run_bass_kernel_spmd(nc, in_maps, list(range(NCORES)))
    return combine(res.results, inputs["bo"])
